# revision 22
# baseline (speedup 1.0000x reference)
"""CostVolume2D Trainium2 Bass kernel (v2: batched DMA, no gpsimd steady-state).

cost[n,d,h,w] = mean_c l[n,c,h,w] * r[n,c,h,w-d]  (0 for w < d)
N=8, C=32, H=256, W=512, D=64.  Data-parallel over batch: core i handles n=i.

vs baseline (2.32 ms):
  - Host pre-casts inputs to bf16 (pads l to width 576): input loads are
    plain HWDGE DMAs; the gpsimd cast path (81%-busy sequencer) is gone.
    Inputs land as [128, 8*Wpad] tiles, partition = 32*j + c (rows
    interleaved mod 4), loaded with 4 big DMAs per 32-row group; per-row
    matmul operands are 32 consecutive partitions (tile_position=(32j,0)).
  - All scratch/output DMAs batched over R=16 rows (256 KB - 1.5 MB per
    DMA, ~250 total instead of ~3300 tiny ones).
  - Strip extraction rides a DRAM scratch shear: band strip k of row-tile
    t lives at scr[t*65536 + 512*p + 256*k + n'], so the strip gather
    (n' = p + d) is the 3-dim AP [[513,128],[65536,R],[1,64]].  DMA APs
    support at most 3 dims, which dictates most layout choices here.
  - [128,128] strip blocks are transposed on TensorE (identity matmul)
    instead of 512 serial xbar-DMA transposes.
  - Output is [h', d, w] (h'-major) so each batch's stores touch a
    compact disjoint byte range (Tile serializes overlapping DRAM
    intervals); host transposes to [d, h, w].

Per (h, wp): M[v,w] = sum_c r[c,v] l[c,w] via 2 TensorE matmuls
(stationary r[c, wb:wb+128], moving l[c, wb:wb+192], wb = (2wp+k)*128);
PSUM band [128, 384] scaled by 1/C into bf16 batch tile; per-(wp,k)
sheared scratch writes; gather strips[p, (rr,k,d)]; PE-transpose each
[128,128] block to T[(k,d), p]; batched store o2[1+h, d, wb+p+d] (w-shift
spill lands in the 64-col pad; w<d zeros stored from a zero tile, w<0
spilling into the previous row's pad).  Host: transpose + slice + f32.
"""

import numpy as np

_CACHE = {}

C, H, W, D = 32, 256, 512, 64
N_CORES = 8
WLP = W + 64              # padded l width (moving operand)
HP = H + 1                # padded out rows (absorbs h=0 zero-store spill)
WOP = W + 64              # padded out cols (absorbs w-run shift spill)
TSZ = 65536               # scratch elems per (wp, h) tile
R = 16                    # rows per DMA batch
RB = R * 384              # band cols per wp block


def _build():
    import concourse.tile as tile
    from concourse import bacc, mybir
    from concourse.ap import AP

    f32 = mybir.dt.float32
    bf16 = mybir.dt.bfloat16
    i32 = mybir.dt.int32

    nc = bacc.Bacc("TRN2", target_bir_lowering=False, debug=False)
    l_d = nc.dram_tensor("l", [C, H, WLP], bf16, kind="ExternalInput")
    r_d = nc.dram_tensor("r", [C, H, W], bf16, kind="ExternalInput")
    # output: 32 contiguous tiles [(b, wp), 128, (k, rr, d)]; host un-shears
    o_d = nc.dram_tensor("o", [1, (H // R) * 2 * 128 * R * 128], bf16,
                         kind="ExternalOutput")
    # scratch: 8 pieces (c, k) per (b, wp) tile, each [32, R*96] contiguous;
    # PSTRIDE pads each piece so the sheared whole-row gather (which over-
    # reads up to 1567 elems past a row) never touches the next piece.
    PIECE = 32 * R * 96
    PSTRIDE = PIECE + 1536
    scr = nc.dram_tensor("scr", [1, (H // R) * 2 * 8 * PSTRIDE + 1536], bf16,
                         kind="Internal")

    with tile.TileContext(nc) as tc:
        with (
            tc.tile_pool(name="io", bufs=2) as io_pool,
            tc.tile_pool(name="band", bufs=3) as band_pool,
            tc.tile_pool(name="xp", bufs=3) as xp_pool,
            tc.tile_pool(name="psum", bufs=8, space="PSUM") as psum_pool,
        ):
            lt = rt = None
            for b in range(H // R):
                h0 = b * R
                if b % 2 == 0:
                    # 32 rows of l, r: partition 32*j + c = row h0+4g+j,
                    # channel c; free col g*Wpad + w.  4 big DMAs each.
                    lt = io_pool.tile([128, 8 * WLP], bf16, tag="lt")
                    rt = io_pool.tile([128, 8 * W], bf16, tag="rt")
                    for j in range(4):
                        nc.sync.dma_start(lt[32 * j:32 * j + 32, :], AP(
                            l_d.ap().tensor, (h0 + j) * WLP,
                            [[H * WLP, 32], [4 * WLP, 8], [1, WLP]]))
                        nc.scalar.dma_start(rt[32 * j:32 * j + 32, :], AP(
                            r_d.ap().tensor, (h0 + j) * W,
                            [[H * W, 32], [4 * W, 8], [1, W]]))

                band = band_pool.tile([128, 2 * RB], bf16, tag="band")
                for rr in range(R):
                    h = h0 + rr
                    g = (h % 32) // 4
                    j = h % 4
                    lrow = lt[32 * j:32 * j + 32, g * WLP:(g + 1) * WLP]
                    rrow = rt[32 * j:32 * j + 32, g * W:(g + 1) * W]
                    for wp in range(2):
                        psum2 = psum_pool.tile([128, 384], f32, tag="ps")
                        for k in range(2):
                            wb = (2 * wp + k) * 128
                            nc.tensor.matmul(
                                psum2[:, 192 * k:192 * k + 192],
                                rrow[:, wb:wb + 128],
                                lrow[:, wb:wb + 192],
                                start=True, stop=True,
                                tile_position=(32 * j, 0),
                            )
                        dst = band[:, wp * RB + rr * 384:wp * RB + (rr + 1) * 384]
                        if wp == 1:
                            nc.scalar.mul(dst, psum2[:], 1.0 / C)
                        else:
                            nc.vector.tensor_scalar_mul(dst, psum2[:], 1.0 / C)

                bandv = band[:].rearrange(
                    "p (wp rr n) -> p wp rr n", wp=2, rr=R, n=384)
                for wp in range(2):
                    # chunked sheared scratch: piece (c, k) holds band
                    # window [32c, 32c+96) of k-strip k for partitions
                    # [32c, 32c+32), laid out contiguously [p', rr, 96].
                    # Writes ~3 descs each; the needed parallelogram only
                    # (25 MiB instead of 48).
                    strips96 = xp_pool.tile([128, 2 * 1536], bf16,
                                            tag="strips")
                    for c in range(4):
                        for k in range(2):
                            pb = ((b * 2 + wp) * 8 + c * 2 + k) * PSTRIDE
                            eng = nc.sync if (c + k) % 2 == 0 else nc.scalar
                            eng.dma_start(AP(
                                scr.ap().tensor, pb,
                                [[R * 96, 32], [96, R], [1, 96]]),
                                bandv[32 * c:32 * c + 32, wp, :,
                                      192 * k + 32 * c:192 * k + 32 * c + 96])
                            # whole-sheared-row gather: reading 1536 elems
                            # from 1537*p' makes the within-row shift cancel:
                            # strips96[32c+p', 1536k + 96rr + d] = strip d
                            eng.dma_start(
                                strips96[32 * c:32 * c + 32,
                                         1536 * k:1536 * (k + 1)],
                                AP(scr.ap().tensor, pb, [[1537, 32], [1, 1536]]))

                    # DVE stream-transpose of the logical [p, (k, rr, d<64)]
                    # view: every 32x32 block transposed in place; the
                    # host's un-shuffle indexes around block positions.
                    sv = strips96[:].rearrange(
                        "p (k rr j) -> p k rr j", k=2, rr=R, j=96)
                    tst = xp_pool.tile([128, R * 128], bf16, tag="tst")
                    tv = tst[:].rearrange(
                        "p (k rr j) -> p k rr j", k=2, rr=R, j=64)
                    nc.vector.transpose(tv, sv[:, :, :, 0:64])

                    # contiguous store of the block-transposed tile; the
                    # host un-shears (w = wp*256 + 128k + p + d) and fills
                    # the w < d zero triangle.
                    eng = nc.sync if wp == 1 else nc.scalar
                    eng.dma_start(AP(
                        o_d.ap().tensor, (b * 2 + wp) * (128 * R * 128),
                        [[R * 128, 128], [1, R * 128]]), tst[:])
    nc.compile()
    return nc


def _get_nc():
    if "nc" not in _CACHE:
        _CACHE["nc"] = _build()
    return _CACHE["nc"]


def _in_maps(l_fmap, r_fmap):
    import ml_dtypes

    bf = ml_dtypes.bfloat16
    l_pad = np.zeros((N_CORES, C, H, WLP), dtype=bf)
    l_pad[..., :W] = l_fmap.astype(bf)
    r_bf = np.ascontiguousarray(r_fmap.astype(bf))
    return [{"l": l_pad[i], "r": r_bf[i]} for i in range(N_CORES)]


def kernel(l_fmap, r_fmap, use_naive, max_disp):
    from concourse.bass_utils import run_bass_kernel_spmd

    l_fmap = np.asarray(l_fmap, dtype=np.float32)
    r_fmap = np.asarray(r_fmap, dtype=np.float32)
    assert int(max_disp) == D, f"kernel hardcoded for max_disp={D}"
    n, c, h, w = l_fmap.shape
    assert (n, c, h, w) == (N_CORES, C, H, W)

    nc = _get_nc()
    in_maps = _in_maps(l_fmap, r_fmap)
    res = run_bass_kernel_spmd(nc, in_maps, core_ids=list(range(N_CORES)))
    # un-shuffle the 32x32-block-transposed strips:
    # o[(b,wp), 32bi+a, k*1024 + rr*64 + 32dj + bs]
    #   = cost[32dj+a, 16b+rr, wp*256+128k+32bi+bs+d]
    arr = np.stack([np.asarray(res.results[i]["o"]) for i in range(N_CORES)])
    arr = arr.reshape(N_CORES, H // R, 2, 4, 32, 2, R, 2, 32)
    # axes: n b wp bi a k rr dj bs -> n (dj a)=d (b rr)=h (wp k bi bs)=w-d
    v = arr.transpose(0, 7, 4, 1, 6, 2, 5, 3, 8).reshape(N_CORES, D, H, W)
    out = np.zeros((N_CORES, D, H, W), dtype=arr.dtype)
    for d in range(D):
        out[:, d, :, d:] = v[:, d, :, :W - d]
    return out.astype(np.float32)
